# revision 9
# baseline (speedup 1.0000x reference)
"""Trainium kernel for nn_Det_AllHist_GLM.

Pipeline (single NeuronCore, bass/Tile):
  phase 1: syn_e = S_e @ C_e.T, syn_i = S_i @ C_i.T as transposed [20, T]
           count maps (PE transposes + bf16 matmuls; spikes/one-hots are
           exactly representable in bf16 via high-half reinterpretation).
  phase 2: causal alpha-kernel filtering of both channels as exact 2nd-order
           IIRs implemented with tensor_tensor_scan (2 scans per channel),
           accumulated into acc[t] = syn(t)+Theta (the "c" term).
  phase 3: sequential pair-march over time: one thresholded scan per 2 steps
           handles the lag-1 self-history tap exactly; dendritic coupling is
           exact via chained v/w IIR scans + a 20x20 PE matmul per pair; the
           199-tap raised-cosine history tail is applied as scatter updates
           (near taps on DVE, far taps on GPSIMD).
  phase 4: s = c + h1*z_{t-1}; P = sigmoid(s); Z cast to f32; DMA out.

kernel(**inputs) takes the full unsharded inputs and returns (Z, P, filters)
exactly like the reference.
"""

import numpy as np

T_DATA = 10000
SUB = 20
E_NO = 2000
I_NO = 500
T_NO = 200
COS_NO = 17
PI = 3.141592653589793

_COMPILED = {}


# ----------------------------------------------------------------------------
# host-side parameter tables (tiny, exact fp32 replicas of the reference math)
# ----------------------------------------------------------------------------

def _cos_basis(T_no):
    f32 = np.float32
    i = np.arange(COS_NO, dtype=f32)
    phi = (np.float32(PI) / np.float32(2.0)) * i
    x = np.arange(T_no, dtype=f32)
    raw = (np.float32(4.0) * np.log(x + np.float32(1.0))).astype(f32)
    b = np.float32(0.5) * np.cos(raw[None, :] - phi[:, None]) + np.float32(0.5)
    mask = (raw[None, :] >= phi[:, None] - np.float32(PI)) & (
        raw[None, :] <= phi[:, None] + np.float32(PI))
    return np.where(mask, b, np.float32(0.0)).astype(f32)


def _make_tables(C_den, Tau_syn, Delta_syn, W_syn, Tau_spk, W_spk, W_hist, Theta):
    f32 = np.float32
    t = np.arange(T_NO, dtype=f32)[None]
    t_e = np.maximum(t - np.exp(Delta_syn[:, 0:1]), 0.0).astype(f32)
    t_i = np.maximum(t - np.exp(Delta_syn[:, 1:2]), 0.0).astype(f32)
    tte = (t_e / np.exp(Tau_syn[:, 0:1])).astype(f32)
    tti = (t_i / np.exp(Tau_syn[:, 1:2])).astype(f32)
    e_kern = (tte * np.exp(-tte) * np.exp(W_syn[:, 0:1])).astype(f32)
    i_kern = (-tti * np.exp(-tti) * np.exp(W_syn[:, 1:2])).astype(f32)
    tt = (np.arange(T_NO, dtype=f32)[None] / np.exp(Tau_spk)[:, None]).astype(f32)
    spk_kern = (tt * np.exp(-tt) * np.exp(W_spk)[:, None]).astype(f32)
    hist_kern = (W_hist @ _cos_basis(T_NO)).astype(f32)
    filters = np.concatenate([e_kern, i_kern, spk_kern, hist_kern], 0)

    def chan(tau_raw, dlt_raw, w_raw, sign):
        # kern[j] = sign * max(j-d,0)/tau * exp(-max(j-d,0)/tau) * exp(w)
        # j>=2 (d<2): kern[j] = A*j*a^j + B*a^j with a=e^{-1/tau},
        # A = sign*e^w/tau*e^{d/tau}, B = -d*A.
        # y_t = k0 x_{t-1} + k1 x_{t-2} + A a^2 S2_{t-3} + (A+B) a^2 S1_{t-3}
        tau = np.exp(tau_raw).astype(f32)
        d = np.exp(dlt_raw).astype(f32)
        assert np.all(d < 2.0), "Delta_syn shift >= 2 unsupported"
        a = np.exp(-1.0 / tau).astype(f32)
        A = (sign * np.exp(w_raw) / tau * np.exp(d / tau)).astype(f32)
        B = (-d * A).astype(f32)
        k0 = (sign * np.maximum(0 - d, 0) / tau * np.exp(-np.maximum(0 - d, 0) / tau)
              * np.exp(w_raw)).astype(f32)
        k1 = (sign * np.maximum(1 - d, 0) / tau * np.exp(-np.maximum(1 - d, 0) / tau)
              * np.exp(w_raw)).astype(f32)
        a2 = (a * a).astype(f32)
        return a, k0, k1, (A * a2).astype(f32), ((A + B) * a2).astype(f32)

    aE, k0e, k1e, Aa2e, ABa2e = chan(Tau_syn[:, 0], Delta_syn[:, 0], W_syn[:, 0], 1.0)
    aI, k0i, k1i, Aa2i, ABa2i = chan(Tau_syn[:, 1], Delta_syn[:, 1], W_syn[:, 1], -1.0)

    tau_s = np.exp(Tau_spk).astype(f32)
    a_spk = np.exp(-1.0 / tau_s).astype(f32)
    G = (np.exp(W_spk) / tau_s).astype(f32)
    Dmat = (C_den * G[None, :]).astype(f32)       # dend = Dmat @ u_{t-1}
    DT = np.ascontiguousarray(Dmat.T)             # lhsT for the PE matmul

    h1 = hist_kern[:, 0].astype(f32)
    HT = np.ascontiguousarray(hist_kern[:, 1:]).astype(f32)   # [20, 199] tail

    # PAR columns: h1neg, a_spk, aE,k0e,k1e,Aa2e,ABa2e, aI,k0i,k1i,Aa2i,ABa2i, Theta, h1
    PAR = np.stack([-h1, a_spk, aE, k0e, k1e, Aa2e, ABa2e,
                    aI, k0i, k1i, Aa2i, ABa2i, Theta.astype(f32), h1], 1).astype(f32)
    return PAR, HT, DT, filters


# ----------------------------------------------------------------------------
# device program
# ----------------------------------------------------------------------------

def _build_program(T):
    import concourse.mybir as mybir
    import concourse.tile as tile
    from concourse import bacc
    from concourse.masks import make_identity

    f32 = mybir.dt.float32
    bf16 = mybir.dt.bfloat16
    AL = mybir.AluOpType
    AF = mybir.ActivationFunctionType

    nc = bacc.Bacc("TRN2", debug=False, num_devices=1)
    Se = nc.dram_tensor("Se", [T, E_NO], f32, kind="ExternalInput")
    Si = nc.dram_tensor("Si", [T, I_NO], f32, kind="ExternalInput")
    Ce = nc.dram_tensor("Ce", [SUB, E_NO], f32, kind="ExternalInput")
    Ci = nc.dram_tensor("Ci", [SUB, I_NO], f32, kind="ExternalInput")
    PARt = nc.dram_tensor("PAR", [SUB, 14], f32, kind="ExternalInput")
    HTt = nc.dram_tensor("HT", [SUB, T_NO - 1], f32, kind="ExternalInput")
    DTt = nc.dram_tensor("DT", [SUB, SUB], f32, kind="ExternalInput")
    OZ = nc.dram_tensor("OZ", [SUB, T], f32, kind="ExternalOutput")
    OP = nc.dram_tensor("OP", [SUB, T], f32, kind="ExternalOutput")

    NPAIR = T // 2
    ECH = 16                   # 16 chunks of 125
    ICH = 4                    # 4 chunks of 125

    with tile.TileContext(nc) as tc:
        with tc.tile_pool(name="const", bufs=1) as cpool, \
             tc.tile_pool(name="big", bufs=1) as bigp, \
             tc.tile_pool(name="pers", bufs=1) as pers, \
             tc.tile_pool(name="ps_small", bufs=2, space="PSUM") as psS, \
             tc.tile_pool(name="vw", bufs=3) as vwp:

            # ---- constants ----
            PAR = cpool.tile([SUB, 14], f32)
            nc.sync.dma_start(out=PAR, in_=PARt.ap())
            HT = cpool.tile([SUB, T_NO - 1], f32)
            nc.sync.dma_start(out=HT, in_=HTt.ap())
            DT = cpool.tile([SUB, SUB], f32)
            nc.sync.dma_start(out=DT, in_=DTt.ap())
            identB = cpool.tile([128, 128], bf16)
            make_identity(nc, identB)
            nh1p = cpool.tile([SUB, 2], f32)   # -h1 replicated to 2 cols
            nc.vector.tensor_copy(out=nh1p, in_=PAR[:, 0:1].broadcast_to([SUB, 2]))
            aspk = PAR[:, 1:2]

            # persistent big state
            accN = pers.tile([SUB, T + 2 * T_NO], f32)     # +c accumulator
            zpad = pers.tile([SUB, T + 1], bf16)           # z with leading 0 col

            nc.vector.memset(accN, 0.0)
            nc.vector.memset(zpad, 0.0)

            synctx = tc.tile_pool(name="syn", bufs=1)
            synp = synctx.__enter__()
            synE = synp.tile([SUB, T], bf16)
            synI = synp.tile([SUB, T], bf16)

            # ---- phase 1: synapse count maps ----
            with tc.tile_pool(name="p1", bufs=2) as p1, \
                 tc.tile_pool(name="p1t", bufs=3) as p1t, \
                 tc.tile_pool(name="ps1", bufs=2, space="PSUM") as ps1, \
                 tc.tile_pool(name="ps1a", bufs=1, space="PSUM") as ps1a:
                # transposed one-hot maps, bf16 (values 0/1 -> high half trick)
                CeF = p1.tile([SUB, E_NO], f32, tag="cef")
                nc.sync.dma_start(out=CeF, in_=Ce.ap())
                CiF = p1.tile([SUB, I_NO], f32, tag="cif")
                nc.sync.dma_start(out=CiF, in_=Ci.ap())
                CeV = CeF.bitcast(bf16)   # [20, 2*E_NO]
                CiV = CiF.bitcast(bf16)
                CeT = cpool.tile([125, SUB * ECH], bf16)
                CiT = cpool.tile([125, SUB * ICH], bf16)
                identS = cpool.tile([SUB, SUB], bf16)
                make_identity(nc, identS)
                for j in range(ECH):
                    pst = ps1.tile([125, SUB], bf16, tag="ct")
                    nc.tensor.transpose(
                        pst, CeV[:, 2 * 125 * j + 1:2 * 125 * (j + 1):2], identS)
                    nc.scalar.copy(out=CeT[:, SUB * j:SUB * (j + 1)], in_=pst)
                for j in range(ICH):
                    pst = ps1.tile([125, SUB], bf16, tag="ct")
                    nc.tensor.transpose(
                        pst, CiV[:, 2 * 125 * j + 1:2 * 125 * (j + 1):2], identS)
                    nc.scalar.copy(out=CiT[:, SUB * j:SUB * (j + 1)], in_=pst)

                ntile = (T + 127) // 128
                for k in range(ntile):
                    r0 = 128 * k
                    rows = min(128, T - r0)
                    se = p1t.tile([128, E_NO], f32, tag="se")
                    nc.sync.dma_start(out=se[:rows], in_=Se.ap()[r0:r0 + rows])
                    si = p1t.tile([128, I_NO], f32, tag="si")
                    nc.sync.dma_start(out=si[:rows], in_=Si.ap()[r0:r0 + rows])
                    seV = se.bitcast(bf16)
                    siV = si.bitcast(bf16)
                    pse = ps1a.tile([SUB, 128], f32, tag="pe")
                    psi = ps1a.tile([SUB, 128], f32, tag="pi")
                    for j in range(ECH):
                        pt = ps1.tile([125, 128], bf16, tag="pt")
                        nc.tensor.transpose(
                            pt[:, :rows],
                            seV[:rows, 2 * 125 * j + 1:2 * 125 * (j + 1):2],
                            identB[:rows, :rows])
                        st = p1.tile([125, 128], bf16, tag="st")
                        nc.scalar.copy(out=st[:, :rows], in_=pt[:, :rows])
                        nc.tensor.matmul(pse[:, :rows],
                                         CeT[:, SUB * j:SUB * (j + 1)],
                                         st[:, :rows],
                                         start=(j == 0), stop=(j == ECH - 1))
                    for j in range(ICH):
                        pt = ps1.tile([125, 128], bf16, tag="pt")
                        nc.tensor.transpose(
                            pt[:, :rows],
                            siV[:rows, 2 * 125 * j + 1:2 * 125 * (j + 1):2],
                            identB[:rows, :rows])
                        st = p1.tile([125, 128], bf16, tag="st")
                        nc.scalar.copy(out=st[:, :rows], in_=pt[:, :rows])
                        nc.tensor.matmul(psi[:, :rows],
                                         CiT[:, SUB * j:SUB * (j + 1)],
                                         st[:, :rows],
                                         start=(j == 0), stop=(j == ICH - 1))
                    nc.scalar.copy(out=synE[:, r0:r0 + rows], in_=pse[:, :rows])
                    nc.scalar.copy(out=synI[:, r0:r0 + rows], in_=psi[:, :rows])

            # ---- phase 2: alpha-kernel IIR filtering into accN ----
            CH = 2500
            NCH = (T + CH - 1) // CH
            with tc.tile_pool(name="s12", bufs=2) as s12:
                for ch in range(2):
                    xv = synE if ch == 0 else synI
                    a_c = PAR[:, 2 + 5 * ch:3 + 5 * ch]
                    k0c = PAR[:, 3 + 5 * ch:4 + 5 * ch]
                    k1c = PAR[:, 4 + 5 * ch:5 + 5 * ch]
                    Aa2 = PAR[:, 5 + 5 * ch:6 + 5 * ch]
                    ABa2 = PAR[:, 6 + 5 * ch:7 + 5 * ch]
                    prev_s1 = None
                    prev_s2 = None
                    for c in range(NCH):
                        c0 = c * CH
                        w_ = min(CH, T - c0)
                        s1 = s12.tile([SUB, CH], f32, tag="s1")
                        s2 = s12.tile([SUB, CH], f32, tag="s2")
                        nc.vector.tensor_tensor_scan(
                            out=s1[:, :w_], data0=a_c.broadcast_to([SUB, w_]),
                            data1=xv[:, c0:c0 + w_],
                            initial=(0.0 if c == 0 else prev_s1[:, CH - 1:CH]),
                            op0=AL.mult, op1=AL.add)
                        nc.vector.tensor_tensor_scan(
                            out=s2[:, :w_], data0=a_c.broadcast_to([SUB, w_]),
                            data1=s1[:, :w_],
                            initial=(0.0 if c == 0 else prev_s2[:, CH - 1:CH]),
                            op0=AL.mult, op1=AL.add)
                        prev_s1, prev_s2 = s1, s2
                        wt = min(w_, T - 3 - c0)
                        nc.vector.scalar_tensor_tensor(
                            out=accN[:, c0 + 3:c0 + 3 + wt], in0=s2[:, :wt],
                            scalar=Aa2, in1=accN[:, c0 + 3:c0 + 3 + wt],
                            op0=AL.mult, op1=AL.add)
                        nc.vector.scalar_tensor_tensor(
                            out=accN[:, c0 + 3:c0 + 3 + wt], in0=s1[:, :wt],
                            scalar=ABa2, in1=accN[:, c0 + 3:c0 + 3 + wt],
                            op0=AL.mult, op1=AL.add)
                    nc.vector.scalar_tensor_tensor(
                        out=accN[:, 1:T], in0=xv[:, 0:T - 1], scalar=k0c,
                        in1=accN[:, 1:T], op0=AL.mult, op1=AL.add)
                    nc.vector.scalar_tensor_tensor(
                        out=accN[:, 2:T], in0=xv[:, 0:T - 2], scalar=k1c,
                        in1=accN[:, 2:T], op0=AL.mult, op1=AL.add)
            nc.vector.tensor_scalar(
                out=accN[:, 0:T], in0=accN[:, 0:T], scalar1=PAR[:, 12:13],
                scalar2=None, op0=AL.add)
            synctx.__exit__(None, None, None)

            # ---- phase 3: pair march ----
            NEAR = 32   # hist tail taps 1..NEAR-1 on DVE, NEAR..198 on GPSIMD
            prev_v = None
            prev_w = None
            for n in range(NPAIR):
                t = 2 * n
                nc.vector.tensor_tensor_scan(
                    out=zpad[:, t + 1:t + 3], data0=nh1p,
                    data1=accN[:, t:t + 2],
                    initial=(0.0 if n == 0 else zpad[:, t:t + 1]),
                    op0=AL.mult, op1=AL.is_lt)
                v = vwp.tile([SUB, 2], f32, tag="v")
                nc.vector.tensor_tensor_scan(
                    out=v, data0=aspk.broadcast_to([SUB, 2]),
                    data1=zpad[:, t + 1:t + 3],
                    initial=(0.0 if n == 0 else prev_v[:, 1:2]),
                    op0=AL.mult, op1=AL.add)
                w = vwp.tile([SUB, 2], f32, tag="w")
                nc.vector.tensor_tensor_scan(
                    out=w, data0=aspk.broadcast_to([SUB, 2]), data1=v,
                    initial=(0.0 if n == 0 else prev_w[:, 1:2]),
                    op0=AL.mult, op1=AL.add)
                prev_v, prev_w = v, w
                u2 = vwp.tile([SUB, 2], f32, tag="u")
                nc.vector.tensor_tensor(
                    out=u2[:, 0:1], in0=w[:, 1:2], in1=v[:, 1:2], op=AL.subtract)
                nc.vector.tensor_scalar_mul(
                    out=u2[:, 1:2], in0=w[:, 1:2], scalar1=aspk)
                pd = psS.tile([SUB, 2], f32, tag="pd")
                nc.tensor.matmul(pd, DT, u2, start=True, stop=True)
                # hist tail scatter: z_t is zpad col t+1, z_{t+1} is col t+2
                nc.vector.scalar_tensor_tensor(
                    out=accN[:, t + 2:t + 1 + NEAR], in0=HT[:, 0:NEAR - 1],
                    scalar=zpad[:, t + 1:t + 2], in1=accN[:, t + 2:t + 1 + NEAR],
                    op0=AL.mult, op1=AL.add)
                nc.vector.scalar_tensor_tensor(
                    out=accN[:, t + 3:t + 2 + NEAR], in0=HT[:, 0:NEAR - 1],
                    scalar=zpad[:, t + 2:t + 3], in1=accN[:, t + 3:t + 2 + NEAR],
                    op0=AL.mult, op1=AL.add)
                nc.vector.scalar_tensor_tensor(
                    out=accN[:, t + 1 + NEAR:t + 201], in0=HT[:, NEAR - 1:199],
                    scalar=zpad[:, t + 1:t + 2], in1=accN[:, t + 1 + NEAR:t + 201],
                    op0=AL.mult, op1=AL.add)
                nc.vector.scalar_tensor_tensor(
                    out=accN[:, t + 2 + NEAR:t + 202], in0=HT[:, NEAR - 1:199],
                    scalar=zpad[:, t + 2:t + 3], in1=accN[:, t + 2 + NEAR:t + 202],
                    op0=AL.mult, op1=AL.add)
                nc.vector.tensor_tensor(
                    out=accN[:, t + 2:t + 4], in0=accN[:, t + 2:t + 4],
                    in1=pd, op=AL.add)

            # ---- phase 4: outputs ----
            with tc.tile_pool(name="outp", bufs=1) as outp:
                S_sb = outp.tile([SUB, T], f32)
                nc.vector.scalar_tensor_tensor(
                    out=S_sb, in0=zpad[:, 0:T], scalar=PAR[:, 13:14],
                    in1=accN[:, 0:T], op0=AL.mult, op1=AL.add)
                Pt = outp.tile([SUB, T], f32)
                nc.scalar.activation(out=Pt, in_=S_sb, func=AF.Sigmoid)
                nc.sync.dma_start(out=OP.ap(), in_=Pt)
                Zt = outp.tile([SUB, T], f32, tag="S_sb")
                nc.scalar.copy(out=Zt, in_=zpad[:, 1:T + 1])
                nc.sync.dma_start(out=OZ.ap(), in_=Zt)

    nc.compile()
    return nc


def _get_program(T):
    if T not in _COMPILED:
        _COMPILED[T] = _build_program(T)
    return _COMPILED[T]


def run_device(inputs, T=T_DATA):
    from concourse.bass_utils import run_bass_kernel_spmd
    f32 = np.float32
    PAR, HT, DT, filters = _make_tables(
        inputs["C_den"].astype(f32), inputs["Tau_syn"].astype(f32),
        inputs["Delta_syn"].astype(f32), inputs["W_syn"].astype(f32),
        inputs["Tau_spk"].astype(f32), inputs["W_spk"].astype(f32),
        inputs["W_hist"].astype(f32), inputs["Theta"].astype(f32))
    nc = _get_program(T)
    in_map = {
        "Se": np.ascontiguousarray(inputs["S_e"][:T].astype(f32)),
        "Si": np.ascontiguousarray(inputs["S_i"][:T].astype(f32)),
        "Ce": np.ascontiguousarray(inputs["C_syn_e"].astype(f32)),
        "Ci": np.ascontiguousarray(inputs["C_syn_i"].astype(f32)),
        "PAR": PAR, "HT": HT, "DT": DT,
    }
    res = run_bass_kernel_spmd(nc, [in_map], core_ids=[0])
    out = res.results[0]
    Z = np.ascontiguousarray(out["OZ"].T)
    P = np.ascontiguousarray(out["OP"].T)
    return Z, P, filters


def kernel(S_e, S_i, C_den, C_syn_e, C_syn_i, Tau_syn, Delta_syn, W_syn,
           Tau_spk, W_spk, W_hist, Theta):
    inputs = dict(S_e=S_e, S_i=S_i, C_den=C_den, C_syn_e=C_syn_e,
                  C_syn_i=C_syn_i, Tau_syn=Tau_syn, Delta_syn=Delta_syn,
                  W_syn=W_syn, Tau_spk=Tau_spk, W_spk=W_spk, W_hist=W_hist,
                  Theta=Theta)
    return run_device(inputs, T=T_DATA)


# revision 10
# speedup vs baseline: 1341.7120x; 1341.7120x over previous
"""Trainium kernel for nn_Det_AllHist_GLM.

Pipeline (single NeuronCore, bass/Tile):
  phase 1: syn_e = S_e @ C_e.T, syn_i = S_i @ C_i.T as transposed [20, T]
           count maps (PE transposes + bf16 matmuls; spikes/one-hots are
           exactly representable in bf16 via high-half reinterpretation).
  phase 2: causal alpha-kernel filtering of both channels as exact 2nd-order
           IIRs implemented with tensor_tensor_scan (2 scans per channel),
           accumulated into acc[t] = syn(t)+Theta (the "c" term).
  phase 3: sequential pair-march over time: one thresholded scan per 2 steps
           handles the lag-1 self-history tap exactly; dendritic coupling is
           exact via chained v/w IIR scans + a 20x20 PE matmul per pair; the
           199-tap raised-cosine history tail is applied as scatter updates
           (near taps on DVE, far taps on GPSIMD).
  phase 4: s = c + h1*z_{t-1}; P = sigmoid(s); Z cast to f32; DMA out.

kernel(**inputs) takes the full unsharded inputs and returns (Z, P, filters)
exactly like the reference.
"""

import numpy as np

T_DATA = 10000
SUB = 20
E_NO = 2000
I_NO = 500
T_NO = 200
COS_NO = 17
PI = 3.141592653589793

_COMPILED = {}


# ----------------------------------------------------------------------------
# host-side parameter tables (tiny, exact fp32 replicas of the reference math)
# ----------------------------------------------------------------------------

def _cos_basis(T_no):
    f32 = np.float32
    i = np.arange(COS_NO, dtype=f32)
    phi = (np.float32(PI) / np.float32(2.0)) * i
    x = np.arange(T_no, dtype=f32)
    raw = (np.float32(4.0) * np.log(x + np.float32(1.0))).astype(f32)
    b = np.float32(0.5) * np.cos(raw[None, :] - phi[:, None]) + np.float32(0.5)
    mask = (raw[None, :] >= phi[:, None] - np.float32(PI)) & (
        raw[None, :] <= phi[:, None] + np.float32(PI))
    return np.where(mask, b, np.float32(0.0)).astype(f32)


def _make_tables(C_den, Tau_syn, Delta_syn, W_syn, Tau_spk, W_spk, W_hist, Theta):
    f32 = np.float32
    t = np.arange(T_NO, dtype=f32)[None]
    t_e = np.maximum(t - np.exp(Delta_syn[:, 0:1]), 0.0).astype(f32)
    t_i = np.maximum(t - np.exp(Delta_syn[:, 1:2]), 0.0).astype(f32)
    tte = (t_e / np.exp(Tau_syn[:, 0:1])).astype(f32)
    tti = (t_i / np.exp(Tau_syn[:, 1:2])).astype(f32)
    e_kern = (tte * np.exp(-tte) * np.exp(W_syn[:, 0:1])).astype(f32)
    i_kern = (-tti * np.exp(-tti) * np.exp(W_syn[:, 1:2])).astype(f32)
    tt = (np.arange(T_NO, dtype=f32)[None] / np.exp(Tau_spk)[:, None]).astype(f32)
    spk_kern = (tt * np.exp(-tt) * np.exp(W_spk)[:, None]).astype(f32)
    hist_kern = (W_hist @ _cos_basis(T_NO)).astype(f32)
    filters = np.concatenate([e_kern, i_kern, spk_kern, hist_kern], 0)

    def chan(tau_raw, dlt_raw, w_raw, sign):
        # kern[j] = sign * max(j-d,0)/tau * exp(-max(j-d,0)/tau) * exp(w)
        # j>=2 (d<2): kern[j] = A*j*a^j + B*a^j with a=e^{-1/tau},
        # A = sign*e^w/tau*e^{d/tau}, B = -d*A.
        # y_t = k0 x_{t-1} + k1 x_{t-2} + A a^2 S2_{t-3} + (A+B) a^2 S1_{t-3}
        tau = np.exp(tau_raw).astype(f32)
        d = np.exp(dlt_raw).astype(f32)
        assert np.all(d < 2.0), "Delta_syn shift >= 2 unsupported"
        a = np.exp(-1.0 / tau).astype(f32)
        A = (sign * np.exp(w_raw) / tau * np.exp(d / tau)).astype(f32)
        B = (-d * A).astype(f32)
        k0 = (sign * np.maximum(0 - d, 0) / tau * np.exp(-np.maximum(0 - d, 0) / tau)
              * np.exp(w_raw)).astype(f32)
        k1 = (sign * np.maximum(1 - d, 0) / tau * np.exp(-np.maximum(1 - d, 0) / tau)
              * np.exp(w_raw)).astype(f32)
        a2 = (a * a).astype(f32)
        return a, k0, k1, (A * a2).astype(f32), ((A + B) * a2).astype(f32)

    aE, k0e, k1e, Aa2e, ABa2e = chan(Tau_syn[:, 0], Delta_syn[:, 0], W_syn[:, 0], 1.0)
    aI, k0i, k1i, Aa2i, ABa2i = chan(Tau_syn[:, 1], Delta_syn[:, 1], W_syn[:, 1], -1.0)

    tau_s = np.exp(Tau_spk).astype(f32)
    a_spk = np.exp(-1.0 / tau_s).astype(f32)
    G = (np.exp(W_spk) / tau_s).astype(f32)
    Dmat = (C_den * G[None, :]).astype(f32)       # dend = Dmat @ u_{t-1}
    DT = np.ascontiguousarray(Dmat.T)             # lhsT for the PE matmul
    NDT = np.ascontiguousarray((-Dmat).T)
    DTa = np.ascontiguousarray((Dmat * a_spk[None, :]).T)

    h1 = hist_kern[:, 0].astype(f32)
    HT = np.ascontiguousarray(hist_kern[:, 1:]).astype(f32)   # [20, 199] tail

    # PAR columns: h1neg, a_spk, aE,k0e,k1e,Aa2e,ABa2e, aI,k0i,k1i,Aa2i,ABa2i, Theta, h1
    PAR = np.stack([-h1, a_spk, aE, k0e, k1e, Aa2e, ABa2e,
                    aI, k0i, k1i, Aa2i, ABa2i, Theta.astype(f32), h1], 1).astype(f32)
    return PAR, HT, np.concatenate([DT, NDT, DTa], 1), filters


# ----------------------------------------------------------------------------
# device program
# ----------------------------------------------------------------------------

def _build_program(T):
    import concourse.mybir as mybir
    import concourse.tile as tile
    from concourse import bacc
    from concourse.masks import make_identity

    f32 = mybir.dt.float32
    bf16 = mybir.dt.bfloat16
    AL = mybir.AluOpType
    AF = mybir.ActivationFunctionType

    nc = bacc.Bacc("TRN2", debug=False, num_devices=1)
    Se = nc.dram_tensor("Se", [T, E_NO], f32, kind="ExternalInput")
    Si = nc.dram_tensor("Si", [T, I_NO], f32, kind="ExternalInput")
    Ce = nc.dram_tensor("Ce", [SUB, E_NO], f32, kind="ExternalInput")
    Ci = nc.dram_tensor("Ci", [SUB, I_NO], f32, kind="ExternalInput")
    PARt = nc.dram_tensor("PAR", [SUB, 14], f32, kind="ExternalInput")
    HTt = nc.dram_tensor("HT", [SUB, T_NO - 1], f32, kind="ExternalInput")
    DTt = nc.dram_tensor("DT", [SUB, 3 * SUB], f32, kind="ExternalInput")
    OZ = nc.dram_tensor("OZ", [SUB, T], f32, kind="ExternalOutput")
    OP = nc.dram_tensor("OP", [SUB, T], f32, kind="ExternalOutput")

    NPAIR = T // 2
    ECH = 16                   # 16 chunks of 125
    ICH = 4                    # 4 chunks of 125

    with tile.TileContext(nc) as tc:
        with tc.tile_pool(name="const", bufs=1) as cpool, \
             tc.tile_pool(name="big", bufs=1) as bigp, \
             tc.tile_pool(name="pers", bufs=1) as pers, \
             tc.tile_pool(name="ps_small", bufs=2, space="PSUM") as psS, \
             tc.tile_pool(name="vw", bufs=3) as vwp:

            # ---- constants ----
            PAR = cpool.tile([SUB, 14], f32)
            nc.sync.dma_start(out=PAR, in_=PARt.ap())
            HT = cpool.tile([SUB, T_NO - 1], f32)
            nc.sync.dma_start(out=HT, in_=HTt.ap())
            DT = cpool.tile([SUB, 3 * SUB], f32)
            nc.sync.dma_start(out=DT, in_=DTt.ap())
            identB = cpool.tile([128, 128], bf16)
            make_identity(nc, identB)
            nh1p = cpool.tile([SUB, 2], f32)   # -h1 replicated to 2 cols
            nc.vector.tensor_copy(out=nh1p, in_=PAR[:, 0:1].broadcast_to([SUB, 2]))
            aspk = PAR[:, 1:2]

            # persistent big state
            accN = pers.tile([SUB, T + 2 * T_NO], f32)     # +c accumulator
            zpad = pers.tile([SUB, T + 1], bf16)           # z with leading 0 col

            nc.vector.memset(accN, 0.0)
            nc.vector.memset(zpad, 0.0)

            synctx = tc.tile_pool(name="syn", bufs=1)
            synp = synctx.__enter__()
            synE = synp.tile([SUB, T], bf16)
            synI = synp.tile([SUB, T], bf16)

            # ---- phase 1: synapse count maps ----
            with tc.tile_pool(name="p1", bufs=2) as p1, \
                 tc.tile_pool(name="p1t", bufs=3) as p1t, \
                 tc.tile_pool(name="ps1", bufs=2, space="PSUM") as ps1, \
                 tc.tile_pool(name="ps1a", bufs=1, space="PSUM") as ps1a:
                # transposed one-hot maps, bf16 (values 0/1 -> high half trick)
                CeF = p1.tile([SUB, E_NO], f32, tag="cef")
                nc.sync.dma_start(out=CeF, in_=Ce.ap())
                CiF = p1.tile([SUB, I_NO], f32, tag="cif")
                nc.sync.dma_start(out=CiF, in_=Ci.ap())
                CeV = CeF.bitcast(bf16)   # [20, 2*E_NO]
                CiV = CiF.bitcast(bf16)
                CeT = cpool.tile([125, SUB * ECH], bf16)
                CiT = cpool.tile([125, SUB * ICH], bf16)
                identS = cpool.tile([SUB, SUB], bf16)
                make_identity(nc, identS)
                for j in range(ECH):
                    pst = ps1.tile([125, SUB], bf16, tag="ct")
                    nc.tensor.transpose(
                        pst, CeV[:, 2 * 125 * j + 1:2 * 125 * (j + 1):2], identS)
                    nc.scalar.copy(out=CeT[:, SUB * j:SUB * (j + 1)], in_=pst)
                for j in range(ICH):
                    pst = ps1.tile([125, SUB], bf16, tag="ct")
                    nc.tensor.transpose(
                        pst, CiV[:, 2 * 125 * j + 1:2 * 125 * (j + 1):2], identS)
                    nc.scalar.copy(out=CiT[:, SUB * j:SUB * (j + 1)], in_=pst)

                ntile = (T + 127) // 128
                for k in range(ntile):
                    r0 = 128 * k
                    rows = min(128, T - r0)
                    se = p1t.tile([128, E_NO], f32, tag="se")
                    nc.sync.dma_start(out=se[:rows], in_=Se.ap()[r0:r0 + rows])
                    si = p1t.tile([128, I_NO], f32, tag="si")
                    nc.sync.dma_start(out=si[:rows], in_=Si.ap()[r0:r0 + rows])
                    seV = se.bitcast(bf16)
                    siV = si.bitcast(bf16)
                    pse = ps1a.tile([SUB, 128], f32, tag="pe")
                    psi = ps1a.tile([SUB, 128], f32, tag="pi")
                    for j in range(ECH):
                        pt = ps1.tile([125, 128], bf16, tag="pt")
                        nc.tensor.transpose(
                            pt[:, :rows],
                            seV[:rows, 2 * 125 * j + 1:2 * 125 * (j + 1):2],
                            identB[:rows, :rows])
                        st = p1.tile([125, 128], bf16, tag="st")
                        nc.scalar.copy(out=st[:, :rows], in_=pt[:, :rows])
                        nc.tensor.matmul(pse[:, :rows],
                                         CeT[:, SUB * j:SUB * (j + 1)],
                                         st[:, :rows],
                                         start=(j == 0), stop=(j == ECH - 1))
                    for j in range(ICH):
                        pt = ps1.tile([125, 128], bf16, tag="pt")
                        nc.tensor.transpose(
                            pt[:, :rows],
                            siV[:rows, 2 * 125 * j + 1:2 * 125 * (j + 1):2],
                            identB[:rows, :rows])
                        st = p1.tile([125, 128], bf16, tag="st")
                        nc.scalar.copy(out=st[:, :rows], in_=pt[:, :rows])
                        nc.tensor.matmul(psi[:, :rows],
                                         CiT[:, SUB * j:SUB * (j + 1)],
                                         st[:, :rows],
                                         start=(j == 0), stop=(j == ICH - 1))
                    nc.scalar.copy(out=synE[:, r0:r0 + rows], in_=pse[:, :rows])
                    nc.scalar.copy(out=synI[:, r0:r0 + rows], in_=psi[:, :rows])

            # ---- phase 2: alpha-kernel IIR filtering into accN ----
            CH = 2500
            NCH = (T + CH - 1) // CH
            with tc.tile_pool(name="s12", bufs=2) as s12:
                for ch in range(2):
                    xv = synE if ch == 0 else synI
                    a_c = PAR[:, 2 + 5 * ch:3 + 5 * ch]
                    k0c = PAR[:, 3 + 5 * ch:4 + 5 * ch]
                    k1c = PAR[:, 4 + 5 * ch:5 + 5 * ch]
                    Aa2 = PAR[:, 5 + 5 * ch:6 + 5 * ch]
                    ABa2 = PAR[:, 6 + 5 * ch:7 + 5 * ch]
                    prev_s1 = None
                    prev_s2 = None
                    for c in range(NCH):
                        c0 = c * CH
                        w_ = min(CH, T - c0)
                        s1 = s12.tile([SUB, CH], f32, tag="s1")
                        s2 = s12.tile([SUB, CH], f32, tag="s2")
                        nc.vector.tensor_tensor_scan(
                            out=s1[:, :w_], data0=a_c.broadcast_to([SUB, w_]),
                            data1=xv[:, c0:c0 + w_],
                            initial=(0.0 if c == 0 else prev_s1[:, CH - 1:CH]),
                            op0=AL.mult, op1=AL.add)
                        nc.vector.tensor_tensor_scan(
                            out=s2[:, :w_], data0=a_c.broadcast_to([SUB, w_]),
                            data1=s1[:, :w_],
                            initial=(0.0 if c == 0 else prev_s2[:, CH - 1:CH]),
                            op0=AL.mult, op1=AL.add)
                        prev_s1, prev_s2 = s1, s2
                        wt = min(w_, T - 3 - c0)
                        nc.vector.scalar_tensor_tensor(
                            out=accN[:, c0 + 3:c0 + 3 + wt], in0=s2[:, :wt],
                            scalar=Aa2, in1=accN[:, c0 + 3:c0 + 3 + wt],
                            op0=AL.mult, op1=AL.add)
                        nc.vector.scalar_tensor_tensor(
                            out=accN[:, c0 + 3:c0 + 3 + wt], in0=s1[:, :wt],
                            scalar=ABa2, in1=accN[:, c0 + 3:c0 + 3 + wt],
                            op0=AL.mult, op1=AL.add)
                    nc.vector.scalar_tensor_tensor(
                        out=accN[:, 1:T], in0=xv[:, 0:T - 1], scalar=k0c,
                        in1=accN[:, 1:T], op0=AL.mult, op1=AL.add)
                    nc.vector.scalar_tensor_tensor(
                        out=accN[:, 2:T], in0=xv[:, 0:T - 2], scalar=k1c,
                        in1=accN[:, 2:T], op0=AL.mult, op1=AL.add)
            nc.vector.tensor_scalar(
                out=accN[:, 0:T], in0=accN[:, 0:T], scalar1=PAR[:, 12:13],
                scalar2=None, op0=AL.add)
            synctx.__exit__(None, None, None)

            # ---- phase 3: pair march ----
            NEAR = 32   # hist tail taps 1..NEAR-1 on DVE, NEAR..198 on GPSIMD
            prev_v = None
            prev_w = None
            for n in range(NPAIR):
                t = 2 * n
                nc.vector.tensor_tensor_scan(
                    out=zpad[:, t + 1:t + 3], data0=nh1p,
                    data1=accN[:, t:t + 2],
                    initial=(0.0 if n == 0 else zpad[:, t:t + 1]),
                    op0=AL.mult, op1=AL.is_lt)
                v = vwp.tile([SUB, 2], f32, tag="v")
                nc.vector.tensor_tensor_scan(
                    out=v, data0=aspk.broadcast_to([SUB, 2]),
                    data1=zpad[:, t + 1:t + 3],
                    initial=(0.0 if n == 0 else prev_v[:, 1:2]),
                    op0=AL.mult, op1=AL.add)
                w = vwp.tile([SUB, 2], f32, tag="w")
                nc.vector.tensor_tensor_scan(
                    out=w, data0=aspk.broadcast_to([SUB, 2]), data1=v,
                    initial=(0.0 if n == 0 else prev_w[:, 1:2]),
                    op0=AL.mult, op1=AL.add)
                prev_v, prev_w = v, w
                pd = psS.tile([SUB, 2], f32, tag="pd")
                nc.tensor.matmul(pd[:, 0:1], DT[:, 0:SUB], w[:, 1:2],
                                 start=True, stop=False)
                nc.tensor.matmul(pd[:, 0:1], DT[:, SUB:2 * SUB], v[:, 1:2],
                                 start=False, stop=True)
                nc.tensor.matmul(pd[:, 1:2], DT[:, 2 * SUB:3 * SUB], w[:, 1:2],
                                 start=True, stop=True)
                # hist tail scatter: z_t is zpad col t+1, z_{t+1} is col t+2
                nc.vector.scalar_tensor_tensor(
                    out=accN[:, t + 2:t + 1 + NEAR], in0=HT[:, 0:NEAR - 1],
                    scalar=zpad[:, t + 1:t + 2], in1=accN[:, t + 2:t + 1 + NEAR],
                    op0=AL.mult, op1=AL.add)
                nc.vector.scalar_tensor_tensor(
                    out=accN[:, t + 3:t + 2 + NEAR], in0=HT[:, 0:NEAR - 1],
                    scalar=zpad[:, t + 2:t + 3], in1=accN[:, t + 3:t + 2 + NEAR],
                    op0=AL.mult, op1=AL.add)
                nc.vector.scalar_tensor_tensor(
                    out=accN[:, t + 1 + NEAR:t + 201], in0=HT[:, NEAR - 1:199],
                    scalar=zpad[:, t + 1:t + 2], in1=accN[:, t + 1 + NEAR:t + 201],
                    op0=AL.mult, op1=AL.add)
                nc.vector.scalar_tensor_tensor(
                    out=accN[:, t + 2 + NEAR:t + 202], in0=HT[:, NEAR - 1:199],
                    scalar=zpad[:, t + 2:t + 3], in1=accN[:, t + 2 + NEAR:t + 202],
                    op0=AL.mult, op1=AL.add)
                nc.vector.tensor_tensor(
                    out=accN[:, t + 2:t + 4], in0=accN[:, t + 2:t + 4],
                    in1=pd, op=AL.add)

            # ---- phase 4: outputs ----
            with tc.tile_pool(name="outp", bufs=1) as outp:
                S_sb = outp.tile([SUB, T], f32)
                nc.vector.scalar_tensor_tensor(
                    out=S_sb, in0=zpad[:, 0:T], scalar=PAR[:, 13:14],
                    in1=accN[:, 0:T], op0=AL.mult, op1=AL.add)
                Pt = outp.tile([SUB, T], f32)
                nc.scalar.activation(out=Pt, in_=S_sb, func=AF.Sigmoid)
                nc.sync.dma_start(out=OP.ap(), in_=Pt)
                Zt = outp.tile([SUB, T], f32, tag="S_sb")
                nc.scalar.copy(out=Zt, in_=zpad[:, 1:T + 1])
                nc.sync.dma_start(out=OZ.ap(), in_=Zt)

    nc.compile()
    return nc


def _get_program(T):
    if T not in _COMPILED:
        _COMPILED[T] = _build_program(T)
    return _COMPILED[T]


def run_device(inputs, T=T_DATA):
    from concourse.bass_utils import run_bass_kernel_spmd
    f32 = np.float32
    PAR, HT, DT, filters = _make_tables(
        inputs["C_den"].astype(f32), inputs["Tau_syn"].astype(f32),
        inputs["Delta_syn"].astype(f32), inputs["W_syn"].astype(f32),
        inputs["Tau_spk"].astype(f32), inputs["W_spk"].astype(f32),
        inputs["W_hist"].astype(f32), inputs["Theta"].astype(f32))
    nc = _get_program(T)
    in_map = {
        "Se": np.ascontiguousarray(inputs["S_e"][:T].astype(f32)),
        "Si": np.ascontiguousarray(inputs["S_i"][:T].astype(f32)),
        "Ce": np.ascontiguousarray(inputs["C_syn_e"].astype(f32)),
        "Ci": np.ascontiguousarray(inputs["C_syn_i"].astype(f32)),
        "PAR": PAR, "HT": HT, "DT": DT,
    }
    res = run_bass_kernel_spmd(nc, [in_map], core_ids=[0])
    out = res.results[0]
    Z = np.ascontiguousarray(out["OZ"].T)
    P = np.ascontiguousarray(out["OP"].T)
    return Z, P, filters


def kernel(S_e, S_i, C_den, C_syn_e, C_syn_i, Tau_syn, Delta_syn, W_syn,
           Tau_spk, W_spk, W_hist, Theta):
    inputs = dict(S_e=S_e, S_i=S_i, C_den=C_den, C_syn_e=C_syn_e,
                  C_syn_i=C_syn_i, Tau_syn=Tau_syn, Delta_syn=Delta_syn,
                  W_syn=W_syn, Tau_spk=Tau_spk, W_spk=W_spk, W_hist=W_hist,
                  Theta=Theta)
    return run_device(inputs, T=T_DATA)
